# revision 10
# baseline (speedup 1.0000x reference)
"""Mixtral MoE block (E=8 experts, top-2, H=1024, I=3584) on 8 Trainium2 cores.

Strategy (expert-parallel, per sharding hint):
  - Host computes the router (logits -> softmax -> top-2 -> renormalized
    combine weights). This is 0.04% of the model FLOPs; all heavy compute
    (the expert SwiGLU MLPs, ~90 GFLOP) runs on the NeuronCores.
  - Tokens are gathered per expert on the host (dispatch); core e computes
    expert e's MLP over a capacity-C batch of its tokens:
        hT = silu(W1 xT) * (W3 xT)                      [I, C]
        outT = W2-col-slices^T @ hT, scaled by routing weight   [H, C]
    (token-transposed layouts throughout; zero on-device transposes)
  - Host scatter-adds the per-core weighted outputs back into [T, H]
    (the "all-reduce the combined output" of the hint, done at unshard).
  - Capacity is one 512-token tile; the few tokens of experts loaded past
    C (~1% of token-expert pairs for balanced routing) are computed
    exactly on the host, which is both faster (25% less padded device
    work than capacity 640) and more accurate.

Matmuls run in float32r (reduced-precision fp32 mode of the PE array,
~1.5e-4 matmul rel err, full 1 cycle/row throughput at moving dim >= 256).
"""

import numpy as np

E, TOPK, H, I = 8, 2, 1024, 3584
P = 128
NCORES = 8
KO = H // P          # 8 k-subtiles of the hidden dim
MI = I // P          # 28 chunks of the intermediate dim
MH = H // P          # 8 chunks of the output hidden dim


def _chunks_ge256(c):
    """Split c (multiple of 128, >=512) into pieces <=512, each >=256
    (float32r runs 1 cycle/row only for moving dims >= 256)."""
    out = []
    rem = c
    while rem > 512:
        take = 512 if rem - 512 >= 256 or rem == 1024 else rem - 256
        if rem - take != 0 and rem - take < 256:
            take = rem - 256
        out.append(take)
        rem -= take
    out.append(rem)
    assert sum(out) == c and all(256 <= t <= 512 for t in out), (c, out)
    return out


def _build_program(C, chunks):
    import concourse.bacc as bacc
    import concourse.mybir as mybir
    from concourse.tile import TileContext

    DT = mybir.dt.float32r
    F32 = mybir.dt.float32
    AF = mybir.ActivationFunctionType

    nc = bacc.Bacc("TRN2", target_bir_lowering=False, debug=False,
                   num_devices=NCORES)
    xT_d = nc.dram_tensor("xT", [P, KO, C], DT, kind="ExternalInput")
    w13_d = nc.dram_tensor("w13", [MI, P, 2, KO, P], DT, kind="ExternalInput")
    w2_d = nc.dram_tensor("w2r", [MH, P, MI, P], DT, kind="ExternalInput")
    wgt_d = nc.dram_tensor("wgt", [P, C], F32, kind="ExternalInput")
    out_d = nc.dram_tensor("outT", [H, C], F32, kind="ExternalOutput")

    with TileContext(nc) as tc:
        with tc.tile_pool(name="const", bufs=1) as constp, \
             tc.tile_pool(name="wpool", bufs=5) as wpool, \
             tc.tile_pool(name="w2pool", bufs=3) as w2pool, \
             tc.tile_pool(name="hpool", bufs=1) as hpool, \
             tc.tile_pool(name="tmp", bufs=3) as tmpp, \
             tc.tile_pool(name="outp", bufs=3) as outp, \
             tc.tile_pool(name="psA", bufs=3, space="PSUM") as psA, \
             tc.tile_pool(name="psB", bufs=2, space="PSUM") as psB:

            # Order matters: the first w13 chunk goes on the DMA queues
            # ahead of everything else (HWDGE queues are FIFO — anything
            # issued earlier delays the first matmul's operands).
            wk0 = wpool.tile([P, 2, KO, P], DT, tag="wk", name="wk0")
            nc.sync.dma_start(wk0[:, 0, 0], w13_d[0, :, 0, 0])
            nc.sync.dma_start(wk0[:, 0, 1:], w13_d[0, :, 0, 1:])

            # PE warm-up: dummy matmuls on a zeroed tile while the first
            # operands stream in. Keeps the PE HAM activity monitor busy so
            # the clock gate is released (1.2 -> 2.4 GHz) before the real
            # matmul stream begins, and covers the idle window so it does
            # not re-throttle.
            warm = tmpp.tile([P, P], F32, tag="warm", name="warm")
            nc.gpsimd.memset(warm[:], 0.0)
            psw = psB.tile([P, 512], F32, tag="psb0", name="psw")
            for i in range(32):
                nc.tensor.matmul(psw[:, :P], warm[:], warm[:],
                                 start=(i == 0), stop=(i == 31))
            # xT split per k-chunk so matmul k can start before the full
            # activation tensor has landed.
            xT = constp.tile([P, KO, C], DT)
            for k in range(KO):
                nc.sync.dma_start(xT[:, k], xT_d[:, k])
            nc.sync.dma_start(wk0[:, 1], w13_d[0, :, 1])
            hT = hpool.tile([P, MI, C], DT)

            # ---- stage A: hT[i, t] = silu(g) * u over all I-chunks ----
            for m in range(MI):
                if m == 0:
                    wk = wk0
                else:
                    wk = wpool.tile([P, 2, KO, P], DT, tag="wk", name="wk")
                    nc.sync.dma_start(wk[:], w13_d[m])
                t0 = 0
                for tn in chunks:
                    psg = psA.tile([P, 512], F32, tag="psg", name="psg")[:, :tn]
                    psu = psA.tile([P, 512], F32, tag="psu", name="psu")[:, :tn]
                    for k in range(KO):
                        nc.tensor.matmul(psg, wk[:, 0, k], xT[:, k, t0:t0 + tn],
                                         start=(k == 0), stop=(k == KO - 1))
                    for k in range(KO):
                        nc.tensor.matmul(psu, wk[:, 1, k], xT[:, k, t0:t0 + tn],
                                         start=(k == 0), stop=(k == KO - 1))
                    sg = tmpp.tile([P, 512], F32, tag="sg", name="sg")[:, :tn]
                    nc.scalar.activation(sg, psg, AF.Silu)
                    nc.vector.tensor_mul(hT[:, m, t0:t0 + tn], sg, psu)
                    t0 += tn

            # ---- stage B: outT[h, t] = sum_i w2T[i, h] * hT[i, t], * wgt ----
            wgt = constp.tile([P, C], F32)
            nc.sync.dma_start(wgt[:], wgt_d[:])
            for mh in range(MH):
                w2c = w2pool.tile([P, MI, P], DT, tag="w2c", name="w2c")
                nc.sync.dma_start(w2c[:], w2_d[mh])
                t0 = 0
                outsb = outp.tile([P, C], F32, tag="outsb", name="outsb")
                for j, tn in enumerate(chunks):
                    psb = psB.tile([P, 512], F32, tag=f"psb{j}",
                                   name=f"psb{j}")[:, :tn]
                    for k in range(MI):
                        nc.tensor.matmul(psb, w2c[:, k], hT[:, k, t0:t0 + tn],
                                         start=(k == 0), stop=(k == MI - 1))
                    nc.vector.tensor_mul(outsb[:, t0:t0 + tn], psb,
                                         wgt[:, t0:t0 + tn])
                    t0 += tn
                nc.sync.dma_start(out_d[mh * P:(mh + 1) * P, :], outsb[:])
    nc.compile()
    return nc


def kernel(hidden_states, w_gate, w1, w3, w2, _trace=False):
    from concourse.bass_utils import run_bass_kernel_spmd

    B, S, Hd = hidden_states.shape
    x = np.ascontiguousarray(hidden_states, dtype=np.float32).reshape(-1, Hd)
    T = x.shape[0]

    # ---- routing (host): logits -> softmax -> top-2 -> renormalize ----
    logits = x @ np.asarray(w_gate, dtype=np.float32).T
    zmax = logits.max(-1, keepdims=True)
    ez = np.exp(logits - zmax)
    probs = ez / ez.sum(-1, keepdims=True)
    top2 = np.argpartition(-probs, TOPK - 1, axis=-1)[:, :TOPK]
    topw = np.take_along_axis(probs, top2, -1)
    topw = topw / topw.sum(-1, keepdims=True)

    idx_list, wv_list = [], []
    for eid in range(E):
        tok, kk = np.nonzero(top2 == eid)
        idx_list.append(tok)
        wv_list.append(topw[tok, kk].astype(np.float32))
    maxn = max(len(ix) for ix in idx_list)
    # capacity: one 512 tile when loads are near-balanced (overflow runs on
    # host); scale up for pathological routing.
    C = 512 if maxn <= 640 else max(((maxn + P - 1) // P) * P, 512)
    chunks = _chunks_ge256(C)

    nc = _build_program(C, chunks)

    w1 = np.asarray(w1, dtype=np.float32)
    w3 = np.asarray(w3, dtype=np.float32)
    w2 = np.asarray(w2, dtype=np.float32)

    in_maps = []
    for eid in range(E):
        ix, wv = idx_list[eid][:C], wv_list[eid][:C]
        n = len(ix)
        xg = np.zeros((C, Hd), np.float32)
        xg[:n] = x[ix]
        xTr = np.ascontiguousarray(xg.T.reshape(KO, P, C).transpose(1, 0, 2))
        w1p = w1[eid].reshape(MI, P, KO, P).transpose(0, 3, 2, 1)
        w3p = w3[eid].reshape(MI, P, KO, P).transpose(0, 3, 2, 1)
        w13 = np.ascontiguousarray(np.stack([w1p, w3p], axis=2))
        w2p = np.ascontiguousarray(
            w2[eid].reshape(MH, P, MI, P).transpose(0, 3, 2, 1))
        wg = np.zeros((C,), np.float32)
        wg[:n] = wv
        wgt_rep = np.ascontiguousarray(np.broadcast_to(wg, (P, C)))
        in_maps.append({"xT": xTr, "w13": w13, "w2r": w2p, "wgt": wgt_rep})

    res = run_bass_kernel_spmd(nc, in_maps, core_ids=list(range(NCORES)),
                               trace=_trace)

    y = np.zeros((T, Hd), np.float32)
    for eid in range(E):
        ix = idx_list[eid][:C]
        outT = res.results[eid]["outT"]          # [H, C]
        y[ix] += outT[:, :len(ix)].T
        # overflow tokens past capacity: exact host compute (tiny)
        ov_ix, ov_wv = idx_list[eid][C:], wv_list[eid][C:]
        if len(ov_ix):
            xs = x[ov_ix]
            g = xs @ w1[eid].T
            u = xs @ w3[eid].T
            h = (g / (1.0 + np.exp(-g))) * u
            y[ov_ix] += ov_wv[:, None] * (h @ w2[eid].T)
    y = y.reshape(B, S, Hd)
    if _trace:
        return y, res
    return y


# revision 11
# speedup vs baseline: 1.0160x; 1.0160x over previous
"""Mixtral MoE block (E=8 experts, top-2, H=1024, I=3584) on 8 Trainium2 cores.

Strategy (expert-parallel, per sharding hint):
  - Host computes the router (logits -> softmax -> top-2 -> renormalized
    combine weights). This is 0.04% of the model FLOPs; all heavy compute
    (the expert SwiGLU MLPs, ~90 GFLOP) runs on the NeuronCores.
  - Tokens are gathered per expert on the host (dispatch); core e computes
    expert e's MLP over a capacity-C batch of its tokens:
        hT = silu(W1 xT) * (W3 xT)                      [I, C]
        outT = W2-col-slices^T @ hT, scaled by routing weight   [H, C]
    (token-transposed layouts throughout; zero on-device transposes)
  - Host scatter-adds the per-core weighted outputs back into [T, H]
    (the "all-reduce the combined output" of the hint, done at unshard).
  - Capacity is one 512-token tile; the few tokens of experts loaded past
    C (~1% of token-expert pairs for balanced routing) are computed
    exactly on the host, which is both faster (25% less padded device
    work than capacity 640) and more accurate.

Matmuls run in float32r (reduced-precision fp32 mode of the PE array,
~1.5e-4 matmul rel err, full 1 cycle/row throughput at moving dim >= 256).
"""

import numpy as np

E, TOPK, H, I = 8, 2, 1024, 3584
P = 128
NCORES = 8
KO = H // P          # 8 k-subtiles of the hidden dim
MI = I // P          # 28 chunks of the intermediate dim
MH = H // P          # 8 chunks of the output hidden dim


def _chunks_ge256(c):
    """Split c (multiple of 128, >=512) into pieces <=512, each >=256
    (float32r runs 1 cycle/row only for moving dims >= 256)."""
    out = []
    rem = c
    while rem > 512:
        take = 512 if rem - 512 >= 256 or rem == 1024 else rem - 256
        if rem - take != 0 and rem - take < 256:
            take = rem - 256
        out.append(take)
        rem -= take
    out.append(rem)
    assert sum(out) == c and all(256 <= t <= 512 for t in out), (c, out)
    return out


def _build_program(C, chunks):
    import concourse.bacc as bacc
    import concourse.mybir as mybir
    from concourse.tile import TileContext

    DT = mybir.dt.float32r
    F32 = mybir.dt.float32
    AF = mybir.ActivationFunctionType

    nc = bacc.Bacc("TRN2", target_bir_lowering=False, debug=False,
                   num_devices=NCORES)
    xT_d = nc.dram_tensor("xT", [P, KO, C], DT, kind="ExternalInput")
    w13_d = nc.dram_tensor("w13", [MI, P, 2, KO, P], DT, kind="ExternalInput")
    w2_d = nc.dram_tensor("w2r", [MH, P, MI, P], DT, kind="ExternalInput")
    wgt_d = nc.dram_tensor("wgt", [P, C], F32, kind="ExternalInput")
    out_d = nc.dram_tensor("outT", [H, C], F32, kind="ExternalOutput")

    with TileContext(nc) as tc:
        with tc.tile_pool(name="const", bufs=1) as constp, \
             tc.tile_pool(name="wpool", bufs=5) as wpool, \
             tc.tile_pool(name="w2pool", bufs=3) as w2pool, \
             tc.tile_pool(name="hpool", bufs=1) as hpool, \
             tc.tile_pool(name="tmp", bufs=3) as tmpp, \
             tc.tile_pool(name="outp", bufs=3) as outp, \
             tc.tile_pool(name="psA", bufs=2, space="PSUM") as psA, \
             tc.tile_pool(name="psB", bufs=2, space="PSUM") as psB:

            # Order matters: the first w13 chunk goes on the DMA queues
            # ahead of everything else (HWDGE queues are FIFO — anything
            # issued earlier delays the first matmul's operands).
            wk0 = wpool.tile([P, 2, KO, P], DT, tag="wk", name="wk0")
            nc.sync.dma_start(wk0[:, 0], w13_d[0, :, 0])

            # PE warm-up: dummy matmuls on a zeroed tile while the first
            # operands stream in. Keeps the PE HAM activity monitor busy so
            # the clock gate is released (1.2 -> 2.4 GHz) before the real
            # matmul stream begins, and covers the idle window so it does
            # not re-throttle.
            warm = tmpp.tile([P, P], F32, tag="warm", name="warm")
            nc.gpsimd.memset(warm[:], 0.0)
            psw = psB.tile([P, 512], F32, tag="psb0", name="psw")
            for i in range(32):
                nc.tensor.matmul(psw[:, :P], warm[:], warm[:],
                                 start=(i == 0), stop=(i == 31))
            # xT split per k-chunk so matmul k can start before the full
            # activation tensor has landed.
            xT = constp.tile([P, KO, C], DT)
            for k in range(KO):
                nc.sync.dma_start(xT[:, k], xT_d[:, k])
            nc.sync.dma_start(wk0[:, 1], w13_d[0, :, 1])
            hT = hpool.tile([P, MI, C], DT)

            # ---- stage A: hT[i, t] = silu(g) * u over all I-chunks ----
            for m in range(MI):
                if m == 0:
                    wk = wk0
                else:
                    wk = wpool.tile([P, 2, KO, P], DT, tag="wk", name="wk")
                    nc.sync.dma_start(wk[:], w13_d[m])
                t0 = 0
                for tn in chunks:
                    psg = psA.tile([P, 512], F32, tag="psg", name="psg")[:, :tn]
                    psu = psA.tile([P, 512], F32, tag="psu", name="psu")[:, :tn]
                    for k in range(KO):
                        nc.tensor.matmul(psg, wk[:, 0, k], xT[:, k, t0:t0 + tn],
                                         start=(k == 0), stop=(k == KO - 1))
                    for k in range(KO):
                        nc.tensor.matmul(psu, wk[:, 1, k], xT[:, k, t0:t0 + tn],
                                         start=(k == 0), stop=(k == KO - 1))
                    sg = tmpp.tile([P, 512], F32, tag="sg", name="sg")[:, :tn]
                    nc.scalar.activation(sg, psg, AF.Silu)
                    nc.vector.tensor_mul(hT[:, m, t0:t0 + tn], sg, psu)
                    t0 += tn

            # ---- stage B: outT[h, t] = sum_i w2T[i, h] * hT[i, t], * wgt ----
            wgt = constp.tile([P, C], F32)
            nc.sync.dma_start(wgt[:], wgt_d[:])
            for mh in range(MH):
                w2c = w2pool.tile([P, MI, P], DT, tag="w2c", name="w2c")
                nc.sync.dma_start(w2c[:], w2_d[mh])
                t0 = 0
                outsb = outp.tile([P, C], F32, tag="outsb", name="outsb")
                for j, tn in enumerate(chunks):
                    psb = psB.tile([P, 512], F32, tag=f"psb{j}",
                                   name=f"psb{j}")[:, :tn]
                    for k in range(MI):
                        nc.tensor.matmul(psb, w2c[:, k], hT[:, k, t0:t0 + tn],
                                         start=(k == 0), stop=(k == MI - 1))
                    nc.vector.tensor_mul(outsb[:, t0:t0 + tn], psb,
                                         wgt[:, t0:t0 + tn])
                    t0 += tn
                nc.sync.dma_start(out_d[mh * P:(mh + 1) * P, :], outsb[:])
    nc.compile()
    return nc


def kernel(hidden_states, w_gate, w1, w3, w2, _trace=False):
    from concourse.bass_utils import run_bass_kernel_spmd

    B, S, Hd = hidden_states.shape
    x = np.ascontiguousarray(hidden_states, dtype=np.float32).reshape(-1, Hd)
    T = x.shape[0]

    # ---- routing (host): logits -> softmax -> top-2 -> renormalize ----
    logits = x @ np.asarray(w_gate, dtype=np.float32).T
    zmax = logits.max(-1, keepdims=True)
    ez = np.exp(logits - zmax)
    probs = ez / ez.sum(-1, keepdims=True)
    top2 = np.argpartition(-probs, TOPK - 1, axis=-1)[:, :TOPK]
    topw = np.take_along_axis(probs, top2, -1)
    topw = topw / topw.sum(-1, keepdims=True)

    idx_list, wv_list = [], []
    for eid in range(E):
        tok, kk = np.nonzero(top2 == eid)
        idx_list.append(tok)
        wv_list.append(topw[tok, kk].astype(np.float32))
    maxn = max(len(ix) for ix in idx_list)
    # capacity: one 512 tile when loads are near-balanced (overflow runs on
    # host); scale up for pathological routing.
    C = 512 if maxn <= 640 else max(((maxn + P - 1) // P) * P, 512)
    chunks = _chunks_ge256(C)

    nc = _build_program(C, chunks)

    w1 = np.asarray(w1, dtype=np.float32)
    w3 = np.asarray(w3, dtype=np.float32)
    w2 = np.asarray(w2, dtype=np.float32)

    in_maps = []
    for eid in range(E):
        ix, wv = idx_list[eid][:C], wv_list[eid][:C]
        n = len(ix)
        xg = np.zeros((C, Hd), np.float32)
        xg[:n] = x[ix]
        xTr = np.ascontiguousarray(xg.T.reshape(KO, P, C).transpose(1, 0, 2))
        w1p = w1[eid].reshape(MI, P, KO, P).transpose(0, 3, 2, 1)
        w3p = w3[eid].reshape(MI, P, KO, P).transpose(0, 3, 2, 1)
        w13 = np.ascontiguousarray(np.stack([w1p, w3p], axis=2))
        w2p = np.ascontiguousarray(
            w2[eid].reshape(MH, P, MI, P).transpose(0, 3, 2, 1))
        wg = np.zeros((C,), np.float32)
        wg[:n] = wv
        wgt_rep = np.ascontiguousarray(np.broadcast_to(wg, (P, C)))
        in_maps.append({"xT": xTr, "w13": w13, "w2r": w2p, "wgt": wgt_rep})

    res = run_bass_kernel_spmd(nc, in_maps, core_ids=list(range(NCORES)),
                               trace=_trace)

    y = np.zeros((T, Hd), np.float32)
    for eid in range(E):
        ix = idx_list[eid][:C]
        outT = res.results[eid]["outT"]          # [H, C]
        y[ix] += outT[:, :len(ix)].T
        # overflow tokens past capacity: exact host compute (tiny)
        ov_ix, ov_wv = idx_list[eid][C:], wv_list[eid][C:]
        if len(ov_ix):
            xs = x[ov_ix]
            g = xs @ w1[eid].T
            u = xs @ w3[eid].T
            h = (g / (1.0 + np.exp(-g))) * u
            y[ov_ix] += ov_wv[:, None] * (h @ w2[eid].T)
    y = y.reshape(B, S, Hd)
    if _trace:
        return y, res
    return y
